# revision 1
# baseline (speedup 1.0000x reference)
"""Trainium2 Bass kernel for nn_LogOddsPerformanceTransformer.

Computes, for each element x of Xs:
    s   = log(x) - log(1-x)              (log-odds)
    idx = clip(searchsorted(bins, max(s, bins[0]), 'right') - 1, 0, NB-1)
    out = bins[idx]

bins is a uniform grid (linspace), so binning reduces to an affine floor
done entirely with fused 2-op vector instructions via the magic-number
rounding trick (no fmod, no gather, no division).  With
b0 = bins[0], step = (bins[-1]-bins[0])/(NB-1), inv = 1/step:

    t1 = s*inv + (2^23 + off)        off = -b0*inv - 0.5 (exact for these bins)
    t2 = clip(t1, 2^23, 2^23 + NB-1) # adding 2^23 floors s*inv+off to an int
    t3 = (t2 - 2^22) - (2^22 - b0*inv)   # both subtractions exact -> idx + b0*inv
    out = t3 * step                      # == idx*step + b0 up to 1 ulp

Data parallel over 8 NeuronCores; per core the 524288-element slice is
processed as a sequence of [128 x fsz] tiles (small head/tail tiles to
shorten pipeline ramp and drain).  t2/t3 instructions are greedily
balanced between the vector engine and gpsimd; the final scale always
runs on gpsimd so output DMAs never stall a compute sequencer.
"""

import sys

sys.path.insert(0, "/opt/trn_rl_repo")

from contextlib import ExitStack

import numpy as np

import concourse.bass as bass
import concourse.tile as tile
from concourse import bacc, mybir
from concourse.bass_utils import run_bass_kernel_spmd

N = 4_194_304
NCORES = 8
NPER = N // NCORES  # 524288
P = 128

# --- tunables -------------------------------------------------------------
TILE_SIZES = (256, 512, 512, 1024, 1024, 512, 256)  # sum = 4096
FC = 512  # DVE/pool compute chunk within a tile
ACT_FULL_TILE = False  # Ln at full tile size (fewer, bigger ACT instrs)
T3_POOL_PATTERN = (0, 1)  # cyclic: 1 -> chunk's unbias TS runs on gpsimd
T2_POOL_PATTERN = (0, 0, 0, 1)  # cyclic: 1 -> chunk's clamp TS runs on gpsimd
TT_POOL_PATTERN = (0,)  # cyclic: 1 -> chunk's subtract runs on gpsimd
FINAL_DVE_TAIL = 2  # last k chunks run t3+final on DVE (pool drains the tail)
LAST_OUT_POOL = False  # last chunk: final on pool + pool-issued out DMA (no sem hop)
DMA_IN_ENGINE = "sync"
DMA_OUT_ENGINE = "sync"
OUT_PER_CHUNK = True  # one out DMA per compute chunk instead of per tile
OPOOL_BUFS = 5
TMP_BUFS = 5
# --------------------------------------------------------------------------

f32 = mybir.dt.float32
Alu = mybir.AluOpType
Act = mybir.ActivationFunctionType

_BUILD_CACHE: dict[tuple, object] = {}


def _constants(bins: np.ndarray):
    """Host-side constants; returns None if the fused-exact path can't be
    used for these bins (non-uniform or inexact magic offsets)."""
    b64 = bins.astype(np.float64)
    nb = len(bins)
    step = np.float32((b64[-1] - b64[0]) / (nb - 1))
    inv = np.float32((nb - 1) / (b64[-1] - b64[0]))
    off = np.float32(-b64[0] * (nb - 1) / (b64[-1] - b64[0]) - 0.5)
    M = np.float32(2.0**23)
    C = np.float32(np.float64(M) + np.float64(off))
    M63 = np.float32(np.float64(M) + (nb - 1))
    U2 = np.float32(2.0**22)
    U2b = np.float32(2.0**22 + (np.float64(off) + 0.5))
    uniform = np.allclose(
        np.diff(b64), (b64[-1] - b64[0]) / (nb - 1), rtol=0, atol=1e-5
    )
    exact = (
        float(C) - float(M) == float(off)
        and float(U2b) == 2.0**22 + float(off) + 0.5
        and float(M63) == 2.0**23 + (nb - 1)
    )
    if not (uniform and exact):
        return None
    return tuple(float(v) for v in (step, inv, C, M, M63, U2, U2b))


# rough per-instruction cost estimates (ns) used only for load balancing
def _dve_ts(n):
    return (n / 2 + 58) / 0.96


def _pool_ts(n):
    return 1579.0 * n / 1024.0


def _build(step, inv, C, M, M63, U2, U2b):
    assert sum(TILE_SIZES) * P == NPER
    NT = len(TILE_SIZES)
    nc = bacc.Bacc("TRN2", target_bir_lowering=False, debug=False)
    xs = [
        nc.dram_tensor(f"xs{i}", [P, fsz], f32, kind="ExternalInput").ap()
        for i, fsz in enumerate(TILE_SIZES)
    ]
    outs = [
        nc.dram_tensor(f"out{i}", [P, fsz], f32, kind="ExternalOutput").ap()
        for i, fsz in enumerate(TILE_SIZES)
    ]

    with tile.TileContext(nc) as tc, ExitStack() as ctx:
        # bufs is per-tag: each x{i} tag is used exactly once, so 1 buf each
        xpool = ctx.enter_context(tc.tile_pool(name="xpool", bufs=1))
        opool = ctx.enter_context(tc.tile_pool(name="opool", bufs=OPOOL_BUFS))
        tmp = ctx.enter_context(tc.tile_pool(name="tmp", bufs=TMP_BUFS))
        dma_in = getattr(nc, DMA_IN_ENGINE)
        dma_out = getattr(nc, DMA_OUT_ENGINE)

        # all input DMAs issued first (high priority) so the out DMAs --
        # which block their sequencer until compute finishes -- never
        # starve later input tiles
        x_tiles = []
        with tc.high_priority():
            for i, fsz in enumerate(TILE_SIZES):
                x = xpool.tile([P, fsz], f32, tag=f"x{i}")
                dma_in.dma_start(x[:], xs[i][:])
                x_tiles.append(x)

        total_chunks = sum(-(-fsz // FC) for fsz in TILE_SIZES)
        chunk_idx = 0
        for i, fsz in enumerate(TILE_SIZES):
            x = x_tiles[i]
            o = opool.tile([P, fsz], f32, tag="o")
            a = tmp.tile([P, fsz], f32, tag="a")
            b = tmp.tile([P, fsz], f32, tag="b")
            if ACT_FULL_TILE:
                nc.scalar.activation(a[:], x[:], Act.Ln)
                nc.scalar.activation(b[:], x[:], Act.Ln, 1.0, -1.0)
            else:
                off = 0
                while off < fsz:
                    fa = min(FC, fsz - off)
                    sla = (slice(None), slice(off, off + fa))
                    nc.scalar.activation(a[sla], x[sla], Act.Ln)
                    nc.scalar.activation(b[sla], x[sla], Act.Ln, 1.0, -1.0)
                    off += fa
            off = 0
            while off < fsz:
                fc = min(FC, fsz - off)
                sl = (slice(None), slice(off, off + fc))
                s = tmp.tile([P, fc], f32, tag="s")
                tt_eng = (
                    nc.gpsimd
                    if TT_POOL_PATTERN[chunk_idx % len(TT_POOL_PATTERN)]
                    else nc.vector
                )
                tt_eng.tensor_sub(s[:], a[sl], b[sl])
                t1 = tmp.tile([P, fc], f32, tag="t1")
                nc.vector.tensor_scalar(t1[:], s[:], inv, C, Alu.mult, Alu.add)
                tail = chunk_idx >= total_chunks - FINAL_DVE_TAIL
                t2 = tmp.tile([P, fc], f32, tag="t2")
                t2_eng = (
                    nc.gpsimd
                    if (T2_POOL_PATTERN[chunk_idx % len(T2_POOL_PATTERN)] and not tail)
                    else nc.vector
                )
                t2_eng.tensor_scalar(t2[:], t1[:], M, M63, Alu.max, Alu.min)
                t3 = tmp.tile([P, fc], f32, tag="t3")
                t3_eng = (
                    nc.gpsimd
                    if (T3_POOL_PATTERN[chunk_idx % len(T3_POOL_PATTERN)] and not tail)
                    else nc.vector
                )
                last = chunk_idx == total_chunks - 1
                chunk_idx += 1
                t3_eng.tensor_scalar(t3[:], t2[:], U2, U2b, Alu.subtract, Alu.subtract)
                if last and LAST_OUT_POOL:
                    final_eng = nc.gpsimd
                elif tail:
                    final_eng = nc.vector
                else:
                    final_eng = nc.gpsimd
                final_eng.tensor_scalar(o[sl], t3[:], step, None, Alu.mult)
                if OUT_PER_CHUNK:
                    eng = nc.gpsimd if (last and LAST_OUT_POOL) else dma_out
                    eng.dma_start(outs[i][sl], o[sl])
                off += fc
            if not OUT_PER_CHUNK:
                dma_out.dma_start(outs[i][:], o[:])

    nc.compile()
    return nc


def build(bins: np.ndarray):
    key = _constants(bins)
    if key is None:
        raise NotImplementedError("non-uniform bins not supported by this kernel")
    if key not in _BUILD_CACHE:
        _BUILD_CACHE[key] = _build(*key)
    return _BUILD_CACHE[key]


def make_in_maps(Xs: np.ndarray):
    shards = Xs.reshape(NCORES, NPER)
    in_maps = []
    for c in range(NCORES):
        m = {}
        off = 0
        for i, fsz in enumerate(TILE_SIZES):
            n = P * fsz
            m[f"xs{i}"] = shards[c, off : off + n].reshape(P, fsz)
            off += n
        in_maps.append(m)
    return in_maps


def kernel(Xs: np.ndarray, bins: np.ndarray) -> np.ndarray:
    Xs = np.asarray(Xs, dtype=np.float32)
    bins = np.asarray(bins, dtype=np.float32)
    nc = build(bins)
    res = run_bass_kernel_spmd(nc, make_in_maps(Xs), core_ids=list(range(NCORES)))
    out = np.concatenate(
        [
            np.concatenate([r[f"out{i}"].reshape(-1) for i in range(len(TILE_SIZES))])
            for r in res.results
        ]
    )
    return out.astype(np.float32)



# revision 2
# speedup vs baseline: 1.0096x; 1.0096x over previous
"""Trainium2 Bass kernel v2 for nn_LogOddsPerformanceTransformer.

For each element x:  s = logit(x);  out = bins[clip(floor((s-b0)/step),0,63)]

Post-ACT arithmetic runs in fp16: tensor_scalar gets the DVE 4x perf
mode and the output DMA halves (values round to fp16; norm-rel ~5e-3,
well under the 2e-2 gate).  Magic-number floor in fp16 (1024 has ulp 1):
    t1 = round16(s*inv + C)        -> 1024 + floor(g),  g=(s-b0)/step
    w  = max(t1 - (C+0.5), 1023.5-C)   (f32 scalars; exact on 0.5 grid)
    o  = min(w, 1086.5-C) * step       (single fp16 rounding)

Two modes per column group:
  A: a=Ln(x), b=Ln(1-x) on ACT (fp16 out); s=a-b (TT on DVE 2x or Pool)
  B: r=reciprocal(x) on DVE (f32); s'=Ln(r-1) on ACT via bias AP=-1
     (fp16); the sign of s' folds into -inv.  One ACT pass instead of
     two — B groups go last so the drain isn't gated on a busy ACT.

The plan decouples granularities: fine input DMA segments keep the ACT
ramp fed; ACT instructions are coarse (222-cycle init each); TT/TS/out
run on sub-chunks for smooth downstream cadence and early outs.

Data parallel over 8 cores; per core [128 x 4096] f32 in, fp16 out,
single DRAM tensors, slice DMAs, full-width SBUF stage buffers.
"""

import sys

sys.path.insert(0, "/opt/trn_rl_repo")

from contextlib import ExitStack

import numpy as np

import concourse.bass as bass
import concourse.tile as tile
from concourse import bacc, mybir
from concourse.bass_utils import run_bass_kernel_spmd

N = 4_194_304
NCORES = 8
NPER = N // NCORES  # 524288
P = 128
W = NPER // P  # 4096 columns per core

# --- plan -----------------------------------------------------------------
# groups: mode 'A'|'B'; cols = ACT instruction span; sub = TT/TS/out chunk
# widths within the group; tt: 'v' DVE / 'p' Pool (A only); w_eng 'v'|'p'
# per-sub engine for the w stage.
PLAN = dict(
    in_segs=(256, 512, 768, 1024, 1024, 512),
    in_eng=("s", "s", "s", "s", "s", "s"),
    groups=(
        dict(mode="A", sub=(256,), tt="v", w_eng=("p",), o_eng=("p",)),
        dict(mode="A", sub=(512,), tt="v", w_eng=("p",), o_eng=("p",)),
        dict(mode="A", sub=(768,), tt="v", w_eng=("p",), o_eng=("p",)),
        dict(mode="B", sub=(1024,), recip=(512, 512)),
        dict(mode="B", sub=(1024,), recip=(512, 512)),
        dict(mode="B", sub=(512,), recip=(512,)),
    ),
    out_segs=(256, 512, 768, 1024, 1024, 512),
    out_eng=("s", "s", "s", "s", "s", "s"),
)
# --------------------------------------------------------------------------

f32 = mybir.dt.float32
f16 = mybir.dt.float16
Alu = mybir.AluOpType
Act = mybir.ActivationFunctionType

_BUILD_CACHE: dict[tuple, object] = {}


def _constants(bins: np.ndarray):
    b64 = bins.astype(np.float64)
    nb = len(bins)
    if nb != 64:
        return None
    step = np.float32((b64[-1] - b64[0]) / (nb - 1))
    inv = np.float32((nb - 1) / (b64[-1] - b64[0]))
    # C = 1024 + (-b0*inv - 0.5): the fp16 round of s*inv + C floors g.
    # For linspace(-6,6,64): -b0*inv = 31.5 so C = 1055.0 exactly.
    C = 1024.0 - float(b64[0]) * float(inv) - 0.5
    if C != float(np.float32(C)) or not (1024.0 < C < 1088.0):
        return None
    uniform = np.allclose(np.diff(b64), (b64[-1] - b64[0]) / (nb - 1), rtol=0, atol=1e-5)
    if not uniform:
        return None
    return (float(step), float(inv), C)


def _engine(nc, code):
    return {"s": nc.sync, "v": nc.vector, "p": nc.gpsimd, "a": nc.scalar}[code]


def _build(step, inv, C, plan=None):
    plan = plan or PLAN
    groups = plan["groups"]
    in_segs = plan["in_segs"]
    out_segs = plan["out_segs"]
    in_eng = plan.get("in_eng", ("s",) * len(in_segs))
    out_eng = plan.get("out_eng", ("s",) * len(out_segs))
    gcols = [sum(g["sub"]) for g in groups]
    assert sum(gcols) == W, (sum(gcols), W)
    assert sum(e[1] if isinstance(e, tuple) else e for e in in_segs) == W
    assert sum(out_segs) == W

    nc = bacc.Bacc("TRN2", target_bir_lowering=False, debug=False)
    x_d = nc.dram_tensor("x", [P, W], f32, kind="ExternalInput").ap()
    # 4-D [batch=1, dhi=1, dho=P, n_ctx=W] so kv_writeback can address it;
    # plain DMA outs use o_d4[0, 0] slices.
    o_d4 = nc.dram_tensor("o", [1, 1, P, W], f16, kind="ExternalOutput").ap()
    n_kv = sum(1 for e in out_eng if e == "k")
    kv_sem = nc.alloc_semaphore("kv_out_sem") if n_kv else None

    with tile.TileContext(nc) as tc, ExitStack() as ctx:
        pool = ctx.enter_context(tc.tile_pool(name="pool", bufs=1))

        need_b = any(g["mode"] == "B" for g in groups)
        cm1 = pool.tile([P, 1], f32, tag="cm1")
        nc.gpsimd.memset(cm1[:], -1.0)
        # Dummy 1-col Ln emitted before any DMA: insert_act_table_loads
        # places the 1283ns natural_log table load here, during the DMA
        # ramp, instead of gating the first real activation on it.
        warm = pool.tile([P, 1], f16, tag="warm")
        nc.scalar.activation(warm[:], cm1[:], Act.Ln, 1.0, -1.0)

        x = pool.tile([P, W], f32, tag="x")
        a = pool.tile([P, W], f16, tag="a")
        b = pool.tile([P, W], f16, tag="b")
        r = pool.tile([P, W], f32, tag="r")
        s = pool.tile([P, W], f16, tag="s")
        t1 = pool.tile([P, W], f16, tag="t1")
        w_ = pool.tile([P, W], f16, tag="w")
        o4 = pool.tile([P, 1, 1, W], f16, tag="o")

        # column-offset index tiles for the kv outs, memset early
        kv_idx = {}
        off = 0
        for k, (wd, eng) in enumerate(zip(out_segs, out_eng)):
            if eng == "k":
                iw = pool.tile([P, 1], mybir.dt.int32, tag=f"oidx{k}")
                nc.gpsimd.memset(iw[:], off)
                kv_idx[k] = iw
            off += wd

        # in_segs entries: width (sequential) or (col_offset, width) for an
        # explicit transfer order — the DMA queue order is free even though
        # column ranges are fixed
        segs = []
        off = 0
        for ent in in_segs:
            if isinstance(ent, tuple):
                segs.append(ent)
            else:
                segs.append((off, ent))
                off += ent
        cov = sorted(segs)
        assert cov[0][0] == 0 and all(
            a + w == b for (a, w), (b, _) in zip(cov, cov[1:])
        ) and cov[-1][0] + cov[-1][1] == W, f"in_segs don't tile [0,{W}): {cov}"
        with tc.high_priority():
            for (start, wd), eng in zip(segs, in_eng):
                sl = (slice(None), slice(start, start + wd))
                _engine(nc, eng).dma_start(x[sl], x_d[sl])

        goff = 0
        for g in groups:
            gw = sum(g["sub"])
            gsl = (slice(None), slice(goff, goff + gw))
            if g["mode"] == "A":
                nc.scalar.activation(a[gsl], x[gsl], Act.Ln)
                nc.scalar.activation(b[gsl], x[gsl], Act.Ln, 1.0, -1.0)
            else:
                roff = goff
                for rw in g["recip"]:
                    rsl = (slice(None), slice(roff, roff + rw))
                    nc.vector.reciprocal(r[rsl], x[rsl])
                    roff += rw
                # s' = Ln(r - 1) = -s ; sign folds into -inv below
                nc.scalar.activation(s[gsl], r[gsl], Act.Ln, cm1[:, 0:1])
            off = goff
            for i, wd in enumerate(g["sub"]):
                sl = (slice(None), slice(off, off + wd))
                if g["mode"] == "A":
                    eng = nc.gpsimd if g.get("tt") == "p" else nc.vector
                    eng.tensor_tensor(s[sl], a[sl], b[sl], Alu.subtract)
                    nc.vector.tensor_scalar(t1[sl], s[sl], inv, C, Alu.mult, Alu.add)
                else:
                    nc.vector.tensor_scalar(t1[sl], s[sl], -inv, C, Alu.mult, Alu.add)
                w_engs = g.get("w_eng")
                weng = nc.gpsimd if (w_engs and w_engs[i] == "p") else nc.vector
                o_engs = g.get("o_eng")
                oeng = nc.gpsimd if (o_engs and o_engs[i] == "p") else nc.vector
                # w = max(t1 - (1024+b0i), -b0i); o = min(w, 63-b0i) * step
                # with b0i = C + 0.5 - 1024 (f32 scalars, exact 0.5-grid out)
                weng.tensor_scalar(w_[sl], t1[sl], C + 0.5, 1023.5 - C, Alu.subtract, Alu.max)
                oeng.tensor_scalar(o4[(slice(None), 0, 0) + sl[1:]], w_[sl], 1086.5 - C, step, Alu.min, Alu.mult)
                off += wd
            goff += gw

        off = 0
        for k, (wd, eng) in enumerate(zip(out_segs, out_eng)):
            sl = (slice(None), slice(off, off + wd))
            if eng == "k":
                # pool-prepared descriptors + cheap trigger: the trigger
                # carries the data dependency and skips HWDGE + DGE delay
                in4 = o4[(slice(None), slice(None), slice(None)) + sl[1:]]  # [P,1,1,wd]
                prep = nc.gpsimd.kv_writeback(
                    o_d4, in4, kv_idx[k][:], prepare_only=True, sem=kv_sem
                )
                # Drop the wrapper-added completion inc so tile's DMASW sem
                # becomes on_update[0]: both sims defer slot 0 to the
                # trigger, and tile's epilogue waits on DMASW — giving the
                # true transfer-completion semantics with no extra wait.
                prep.ins.sync_info = mybir.SyncInfo(on_wait=[], on_update=[])
                nc.gpsimd.trigger_dma(count=None)
            else:
                _engine(nc, eng).dma_start(o_d4[(0, 0) + sl], o4[(slice(None), 0, 0) + sl[1:]])
            off += wd

    nc.compile()
    return nc


def _freeze(obj):
    if isinstance(obj, dict):
        return tuple(sorted((k, _freeze(v)) for k, v in obj.items()))
    if isinstance(obj, (list, tuple)):
        return tuple(_freeze(v) for v in obj)
    return obj


def build(bins: np.ndarray, plan=None):
    key = _constants(bins)
    if key is None:
        raise NotImplementedError("bins not supported by this kernel")
    full_key = (key, _freeze(plan))
    if full_key not in _BUILD_CACHE:
        _BUILD_CACHE[full_key] = _build(*key, plan=plan)
    return _BUILD_CACHE[full_key]


def make_in_maps(Xs: np.ndarray):
    shards = Xs.reshape(NCORES, P, W)
    return [{"x": shards[c]} for c in range(NCORES)]


def kernel(Xs: np.ndarray, bins: np.ndarray) -> np.ndarray:
    Xs = np.asarray(Xs, dtype=np.float32)
    bins = np.asarray(bins, dtype=np.float32)
    nc = build(bins)
    res = run_bass_kernel_spmd(nc, make_in_maps(Xs), core_ids=list(range(NCORES)))
    out = np.concatenate([r["o"].reshape(-1) for r in res.results])
    return out.astype(np.float32)


# revision 3
# speedup vs baseline: 1.0227x; 1.0129x over previous
"""Trainium2 Bass kernel v2 for nn_LogOddsPerformanceTransformer.

For each element x:  s = logit(x);  out = bins[clip(floor((s-b0)/step),0,63)]

Post-ACT arithmetic runs in fp16: tensor_scalar gets the DVE 4x perf
mode and the output DMA halves (values round to fp16; norm-rel ~5e-3,
well under the 2e-2 gate).  Magic-number floor in fp16 (1024 has ulp 1):
    t1 = round16(s*inv + C)        -> 1024 + floor(g),  g=(s-b0)/step
    w  = max(t1 - (C+0.5), 1023.5-C)   (f32 scalars; exact on 0.5 grid)
    o  = min(w, 1086.5-C) * step       (single fp16 rounding)

Two modes per column group:
  A: a=Ln(x), b=Ln(1-x) on ACT (fp16 out); s=a-b (TT on DVE 2x or Pool)
  B: r=reciprocal(x) on DVE (f32); s'=Ln(r-1) on ACT via bias AP=-1
     (fp16); the sign of s' folds into -inv.  One ACT pass instead of
     two — B groups go last so the drain isn't gated on a busy ACT.

The plan decouples granularities: fine input DMA segments keep the ACT
ramp fed; ACT instructions are coarse (222-cycle init each); TT/TS/out
run on sub-chunks for smooth downstream cadence and early outs.

Data parallel over 8 cores; per core [128 x 4096] f32 in, fp16 out,
single DRAM tensors, slice DMAs, full-width SBUF stage buffers.
"""

import sys

sys.path.insert(0, "/opt/trn_rl_repo")

from contextlib import ExitStack

import numpy as np

import concourse.bass as bass
import concourse.tile as tile
from concourse import bacc, mybir
from concourse.bass_utils import run_bass_kernel_spmd

N = 4_194_304
NCORES = 8
NPER = N // NCORES  # 524288
P = 128
W = NPER // P  # 4096 columns per core

# --- plan -----------------------------------------------------------------
# groups: mode 'A'|'B'; cols = ACT instruction span; sub = TT/TS/out chunk
# widths within the group; tt: 'v' DVE / 'p' Pool (A only); w_eng 'v'|'p'
# per-sub engine for the w stage.
PLAN = dict(
    in_segs=(256, 512, 768, 512, 512, 512, 512, 512),
    in_eng=("s", "s", "s", "s", "s", "s", "s", "s"),
    groups=(
        dict(mode="A", sub=(256,), tt="v", w_eng=("p",), o_eng=("p",)),
        dict(mode="A", sub=(512,), tt="v", w_eng=("p",), o_eng=("p",)),
        dict(mode="A", sub=(384, 384), tt="v", w_eng=("p", "p"), o_eng=("p", "p")),
        dict(mode="B", sub=(1024,), recip=(512, 512)),
        dict(mode="B", sub=(1024,), recip=(512, 512)),
        dict(mode="B", sub=(512,), recip=(512,)),
    ),
    out_segs=(256, 512, 768, 1024, 1024, 512),
    out_eng=("s", "s", "s", "s", "s", "s"),
)
# --------------------------------------------------------------------------

f32 = mybir.dt.float32
f16 = mybir.dt.float16
Alu = mybir.AluOpType
Act = mybir.ActivationFunctionType

_BUILD_CACHE: dict[tuple, object] = {}


def _constants(bins: np.ndarray):
    b64 = bins.astype(np.float64)
    nb = len(bins)
    if nb != 64:
        return None
    step = np.float32((b64[-1] - b64[0]) / (nb - 1))
    inv = np.float32((nb - 1) / (b64[-1] - b64[0]))
    # C = 1024 + (-b0*inv - 0.5): the fp16 round of s*inv + C floors g.
    # For linspace(-6,6,64): -b0*inv = 31.5 so C = 1055.0 exactly.
    C = 1024.0 - float(b64[0]) * float(inv) - 0.5
    if C != float(np.float32(C)) or not (1024.0 < C < 1088.0):
        return None
    uniform = np.allclose(np.diff(b64), (b64[-1] - b64[0]) / (nb - 1), rtol=0, atol=1e-5)
    if not uniform:
        return None
    return (float(step), float(inv), C)


def _engine(nc, code):
    return {"s": nc.sync, "v": nc.vector, "p": nc.gpsimd, "a": nc.scalar}[code]


def _build(step, inv, C, plan=None):
    plan = plan or PLAN
    groups = plan["groups"]
    in_segs = plan["in_segs"]
    out_segs = plan["out_segs"]
    in_eng = plan.get("in_eng", ("s",) * len(in_segs))
    out_eng = plan.get("out_eng", ("s",) * len(out_segs))
    gcols = [sum(g["sub"]) for g in groups]
    assert sum(gcols) == W, (sum(gcols), W)
    assert sum(e[1] if isinstance(e, tuple) else e for e in in_segs) == W
    assert sum(out_segs) == W

    nc = bacc.Bacc("TRN2", target_bir_lowering=False, debug=False)
    x_d = nc.dram_tensor("x", [P, W], f32, kind="ExternalInput").ap()
    # 4-D [batch=1, dhi=1, dho=P, n_ctx=W] so kv_writeback can address it;
    # plain DMA outs use o_d4[0, 0] slices.
    o_d4 = nc.dram_tensor("o", [1, 1, P, W], f16, kind="ExternalOutput").ap()
    n_kv = sum(1 for e in out_eng if e == "k")
    kv_sem = nc.alloc_semaphore("kv_out_sem") if n_kv else None

    with tile.TileContext(nc) as tc, ExitStack() as ctx:
        pool = ctx.enter_context(tc.tile_pool(name="pool", bufs=1))

        need_b = any(g["mode"] == "B" for g in groups)
        cm1 = pool.tile([P, 1], f32, tag="cm1")
        nc.gpsimd.memset(cm1[:], -1.0)
        # Dummy 1-col Ln emitted before any DMA: insert_act_table_loads
        # places the 1283ns natural_log table load here, during the DMA
        # ramp, instead of gating the first real activation on it.
        warm = pool.tile([P, 1], f16, tag="warm")
        nc.scalar.activation(warm[:], cm1[:], Act.Ln, 1.0, -1.0)

        x = pool.tile([P, W], f32, tag="x")
        a = pool.tile([P, W], f16, tag="a")
        b = pool.tile([P, W], f16, tag="b")
        r = pool.tile([P, W], f32, tag="r")
        s = pool.tile([P, W], f16, tag="s")
        t1 = pool.tile([P, W], f16, tag="t1")
        w_ = pool.tile([P, W], f16, tag="w")
        o4 = pool.tile([P, 1, 1, W], f16, tag="o")

        # column-offset index tiles for the kv outs, memset early
        kv_idx = {}
        off = 0
        for k, (wd, eng) in enumerate(zip(out_segs, out_eng)):
            if eng == "k":
                iw = pool.tile([P, 1], mybir.dt.int32, tag=f"oidx{k}")
                nc.gpsimd.memset(iw[:], off)
                kv_idx[k] = iw
            off += wd

        # in_segs entries: width (sequential) or (col_offset, width) for an
        # explicit transfer order — the DMA queue order is free even though
        # column ranges are fixed
        segs = []
        off = 0
        for ent in in_segs:
            if isinstance(ent, tuple):
                segs.append(ent)
            else:
                segs.append((off, ent))
                off += ent
        cov = sorted(segs)
        assert cov[0][0] == 0 and all(
            a + w == b for (a, w), (b, _) in zip(cov, cov[1:])
        ) and cov[-1][0] + cov[-1][1] == W, f"in_segs don't tile [0,{W}): {cov}"
        with tc.high_priority():
            for (start, wd), eng in zip(segs, in_eng):
                sl = (slice(None), slice(start, start + wd))
                _engine(nc, eng).dma_start(x[sl], x_d[sl])

        goff = 0
        for g in groups:
            gctx = tc.high_priority(offset=g["prio"]) if g.get("prio") else None
            if gctx:
                gctx.__enter__()
            gw = sum(g["sub"])
            gsl = (slice(None), slice(goff, goff + gw))
            if g["mode"] == "A":
                nc.scalar.activation(a[gsl], x[gsl], Act.Ln)
                nc.scalar.activation(b[gsl], x[gsl], Act.Ln, 1.0, -1.0)
            else:
                roff = goff
                for rw in g["recip"]:
                    rsl = (slice(None), slice(roff, roff + rw))
                    nc.vector.reciprocal(r[rsl], x[rsl])
                    roff += rw
                # s' = Ln(r - 1) = -s ; sign folds into -inv below
                nc.scalar.activation(s[gsl], r[gsl], Act.Ln, cm1[:, 0:1])
            off = goff
            for i, wd in enumerate(g["sub"]):
                sl = (slice(None), slice(off, off + wd))
                if g["mode"] == "A":
                    eng = nc.gpsimd if g.get("tt") == "p" else nc.vector
                    eng.tensor_tensor(s[sl], a[sl], b[sl], Alu.subtract)
                    nc.vector.tensor_scalar(t1[sl], s[sl], inv, C, Alu.mult, Alu.add)
                else:
                    nc.vector.tensor_scalar(t1[sl], s[sl], -inv, C, Alu.mult, Alu.add)
                w_engs = g.get("w_eng")
                weng = nc.gpsimd if (w_engs and w_engs[i] == "p") else nc.vector
                o_engs = g.get("o_eng")
                oeng = nc.gpsimd if (o_engs and o_engs[i] == "p") else nc.vector
                # w = max(t1 - (1024+b0i), -b0i); o = min(w, 63-b0i) * step
                # with b0i = C + 0.5 - 1024 (f32 scalars, exact 0.5-grid out)
                weng.tensor_scalar(w_[sl], t1[sl], C + 0.5, 1023.5 - C, Alu.subtract, Alu.max)
                oeng.tensor_scalar(o4[(slice(None), 0, 0) + sl[1:]], w_[sl], 1086.5 - C, step, Alu.min, Alu.mult)
                off += wd
            if gctx:
                gctx.__exit__(None, None, None)
            goff += gw

        off = 0
        for k, (wd, eng) in enumerate(zip(out_segs, out_eng)):
            sl = (slice(None), slice(off, off + wd))
            if eng == "k":
                # pool-prepared descriptors + cheap trigger: the trigger
                # carries the data dependency and skips HWDGE + DGE delay
                in4 = o4[(slice(None), slice(None), slice(None)) + sl[1:]]  # [P,1,1,wd]
                prep = nc.gpsimd.kv_writeback(
                    o_d4, in4, kv_idx[k][:], prepare_only=True, sem=kv_sem
                )
                # Drop the wrapper-added completion inc so tile's DMASW sem
                # becomes on_update[0]: both sims defer slot 0 to the
                # trigger, and tile's epilogue waits on DMASW — giving the
                # true transfer-completion semantics with no extra wait.
                prep.ins.sync_info = mybir.SyncInfo(on_wait=[], on_update=[])
                nc.gpsimd.trigger_dma(count=None)
            else:
                _engine(nc, eng).dma_start(o_d4[(0, 0) + sl], o4[(slice(None), 0, 0) + sl[1:]])
            off += wd

    nc.compile()
    return nc


def _freeze(obj):
    if isinstance(obj, dict):
        return tuple(sorted((k, _freeze(v)) for k, v in obj.items()))
    if isinstance(obj, (list, tuple)):
        return tuple(_freeze(v) for v in obj)
    return obj


def build(bins: np.ndarray, plan=None):
    key = _constants(bins)
    if key is None:
        raise NotImplementedError("bins not supported by this kernel")
    full_key = (key, _freeze(plan))
    if full_key not in _BUILD_CACHE:
        _BUILD_CACHE[full_key] = _build(*key, plan=plan)
    return _BUILD_CACHE[full_key]


def make_in_maps(Xs: np.ndarray):
    shards = Xs.reshape(NCORES, P, W)
    return [{"x": shards[c]} for c in range(NCORES)]


def kernel(Xs: np.ndarray, bins: np.ndarray) -> np.ndarray:
    Xs = np.asarray(Xs, dtype=np.float32)
    bins = np.asarray(bins, dtype=np.float32)
    nc = build(bins)
    res = run_bass_kernel_spmd(nc, make_in_maps(Xs), core_ids=list(range(NCORES)))
    out = np.concatenate([r["o"].reshape(-1) for r in res.results])
    return out.astype(np.float32)


# revision 4
# speedup vs baseline: 1.0243x; 1.0015x over previous
"""Trainium2 Bass kernel v2 for nn_LogOddsPerformanceTransformer.

For each element x:  s = logit(x);  out = bins[clip(floor((s-b0)/step),0,63)]

Post-ACT arithmetic runs in fp16: tensor_scalar gets the DVE 4x perf
mode and the output DMA halves (values round to fp16; norm-rel ~5e-3,
well under the 2e-2 gate).  Magic-number floor in fp16 (1024 has ulp 1):
    t1 = round16(s*inv + C)        -> 1024 + floor(g),  g=(s-b0)/step
    w  = max(t1 - (C+0.5), 1023.5-C)   (f32 scalars; exact on 0.5 grid)
    o  = min(w, 1086.5-C) * step       (single fp16 rounding)

Two modes per column group:
  A: a=Ln(x), b=Ln(1-x) on ACT (fp16 out); s=a-b (TT on DVE 2x or Pool)
  B: r=reciprocal(x) on DVE (f32); s'=Ln(r-1) on ACT via bias AP=-1
     (fp16); the sign of s' folds into -inv.  One ACT pass instead of
     two — B groups go last so the drain isn't gated on a busy ACT.

The plan decouples granularities: fine input DMA segments keep the ACT
ramp fed; ACT instructions are coarse (222-cycle init each); TT/TS/out
run on sub-chunks for smooth downstream cadence and early outs.

Data parallel over 8 cores; per core [128 x 4096] f32 in, fp16 out,
single DRAM tensors, slice DMAs, full-width SBUF stage buffers.
"""

import sys

sys.path.insert(0, "/opt/trn_rl_repo")

from contextlib import ExitStack

import numpy as np

import concourse.bass as bass
import concourse.tile as tile
from concourse import bacc, mybir
from concourse.bass_utils import run_bass_kernel_spmd

N = 4_194_304
NCORES = 8
NPER = N // NCORES  # 524288
P = 128
W = NPER // P  # 4096 columns per core

# --- plan -----------------------------------------------------------------
# groups: mode 'A'|'B'; cols = ACT instruction span; sub = TT/TS/out chunk
# widths within the group; tt: 'v' DVE / 'p' Pool (A only); w_eng 'v'|'p'
# per-sub engine for the w stage.
PLAN = dict(
    in_segs=(256, 512, 384, 384, 512, 512, 512, 512, 512),
    in_eng=("s",) * 9,
    groups=(
        dict(mode="A", sub=(256,), tt="v", w_eng=("p",), o_eng=("p",)),
        dict(mode="A", sub=(512,), tt="v", w_eng=("p",), o_eng=("p",)),
        dict(mode="A", sub=(384, 384), tt="v", w_eng=("p", "p"), o_eng=("p", "p")),
        dict(mode="B", sub=(1024,), recip=(512, 512)),
        dict(mode="B", sub=(1024,), recip=(512, 512)),
        dict(mode="B", sub=(512,), recip=(512,)),
    ),
    out_segs=(256, 512, 768, 1024, 1024, 512),
    out_eng=("s", "s", "s", "s", "s", "s"),
)
# --------------------------------------------------------------------------

f32 = mybir.dt.float32
f16 = mybir.dt.float16
Alu = mybir.AluOpType
Act = mybir.ActivationFunctionType

_BUILD_CACHE: dict[tuple, object] = {}


def _constants(bins: np.ndarray):
    b64 = bins.astype(np.float64)
    nb = len(bins)
    if nb != 64:
        return None
    step = np.float32((b64[-1] - b64[0]) / (nb - 1))
    inv = np.float32((nb - 1) / (b64[-1] - b64[0]))
    # C = 1024 + (-b0*inv - 0.5): the fp16 round of s*inv + C floors g.
    # For linspace(-6,6,64): -b0*inv = 31.5 so C = 1055.0 exactly.
    C = 1024.0 - float(b64[0]) * float(inv) - 0.5
    if C != float(np.float32(C)) or not (1024.0 < C < 1088.0):
        return None
    uniform = np.allclose(np.diff(b64), (b64[-1] - b64[0]) / (nb - 1), rtol=0, atol=1e-5)
    if not uniform:
        return None
    return (float(step), float(inv), C)


def _engine(nc, code):
    return {"s": nc.sync, "v": nc.vector, "p": nc.gpsimd, "a": nc.scalar}[code]


def _build(step, inv, C, plan=None):
    plan = plan or PLAN
    groups = plan["groups"]
    in_segs = plan["in_segs"]
    out_segs = plan["out_segs"]
    in_eng = plan.get("in_eng", ("s",) * len(in_segs))
    out_eng = plan.get("out_eng", ("s",) * len(out_segs))
    gcols = [sum(g["sub"]) for g in groups]
    assert sum(gcols) == W, (sum(gcols), W)
    assert sum(e[1] if isinstance(e, tuple) else e for e in in_segs) == W
    assert sum(out_segs) == W

    nc = bacc.Bacc("TRN2", target_bir_lowering=False, debug=False)
    x_d = nc.dram_tensor("x", [P, W], f32, kind="ExternalInput").ap()
    # 4-D [batch=1, dhi=1, dho=P, n_ctx=W] so kv_writeback can address it;
    # plain DMA outs use o_d4[0, 0] slices.
    o_d4 = nc.dram_tensor("o", [1, 1, P, W], f16, kind="ExternalOutput").ap()
    n_kv = sum(1 for e in out_eng if e == "k")
    kv_sem = nc.alloc_semaphore("kv_out_sem") if n_kv else None

    with tile.TileContext(nc) as tc, ExitStack() as ctx:
        pool = ctx.enter_context(tc.tile_pool(name="pool", bufs=1))

        need_b = any(g["mode"] == "B" for g in groups)
        cm1 = pool.tile([P, 1], f32, tag="cm1")
        nc.gpsimd.memset(cm1[:], -1.0)
        # Dummy 1-col Ln emitted before any DMA: insert_act_table_loads
        # places the 1283ns natural_log table load here, during the DMA
        # ramp, instead of gating the first real activation on it.
        warm = pool.tile([P, 1], f16, tag="warm")
        nc.scalar.activation(warm[:], cm1[:], Act.Ln, 1.0, -1.0)

        x = pool.tile([P, W], f32, tag="x")
        a = pool.tile([P, W], f16, tag="a")
        b = pool.tile([P, W], f16, tag="b")
        r = pool.tile([P, W], f32, tag="r")
        s = pool.tile([P, W], f16, tag="s")
        t1 = pool.tile([P, W], f16, tag="t1")
        w_ = pool.tile([P, W], f16, tag="w")
        o4 = pool.tile([P, 1, 1, W], f16, tag="o")

        # column-offset index tiles for the kv outs, memset early
        kv_idx = {}
        off = 0
        for k, (wd, eng) in enumerate(zip(out_segs, out_eng)):
            if eng == "k":
                iw = pool.tile([P, 1], mybir.dt.int32, tag=f"oidx{k}")
                nc.gpsimd.memset(iw[:], off)
                kv_idx[k] = iw
            off += wd

        # in_segs entries: width (sequential) or (col_offset, width) for an
        # explicit transfer order — the DMA queue order is free even though
        # column ranges are fixed
        segs = []
        off = 0
        for ent in in_segs:
            if isinstance(ent, tuple):
                segs.append(ent)
            else:
                segs.append((off, ent))
                off += ent
        cov = sorted(segs)
        assert cov[0][0] == 0 and all(
            a + w == b for (a, w), (b, _) in zip(cov, cov[1:])
        ) and cov[-1][0] + cov[-1][1] == W, f"in_segs don't tile [0,{W}): {cov}"
        with tc.high_priority():
            for (start, wd), eng in zip(segs, in_eng):
                sl = (slice(None), slice(start, start + wd))
                _engine(nc, eng).dma_start(x[sl], x_d[sl])

        goff = 0
        for g in groups:
            gctx = tc.high_priority(offset=g["prio"]) if g.get("prio") else None
            if gctx:
                gctx.__enter__()
            gw = sum(g["sub"])
            gsl = (slice(None), slice(goff, goff + gw))
            if g["mode"] == "A":
                nc.scalar.activation(a[gsl], x[gsl], Act.Ln)
                nc.scalar.activation(b[gsl], x[gsl], Act.Ln, 1.0, -1.0)
            else:
                roff = goff
                for rw in g["recip"]:
                    rsl = (slice(None), slice(roff, roff + rw))
                    nc.vector.reciprocal(r[rsl], x[rsl])
                    roff += rw
                # s' = Ln(r - 1) = -s ; sign folds into -inv below
                nc.scalar.activation(s[gsl], r[gsl], Act.Ln, cm1[:, 0:1])
            off = goff
            for i, wd in enumerate(g["sub"]):
                sl = (slice(None), slice(off, off + wd))
                if g["mode"] == "A":
                    eng = nc.gpsimd if g.get("tt") == "p" else nc.vector
                    eng.tensor_tensor(s[sl], a[sl], b[sl], Alu.subtract)
                    nc.vector.tensor_scalar(t1[sl], s[sl], inv, C, Alu.mult, Alu.add)
                else:
                    nc.vector.tensor_scalar(t1[sl], s[sl], -inv, C, Alu.mult, Alu.add)
                w_engs = g.get("w_eng")
                weng = nc.gpsimd if (w_engs and w_engs[i] == "p") else nc.vector
                o_engs = g.get("o_eng")
                oeng = nc.gpsimd if (o_engs and o_engs[i] == "p") else nc.vector
                # w = max(t1 - (1024+b0i), -b0i); o = min(w, 63-b0i) * step
                # with b0i = C + 0.5 - 1024 (f32 scalars, exact 0.5-grid out)
                weng.tensor_scalar(w_[sl], t1[sl], C + 0.5, 1023.5 - C, Alu.subtract, Alu.max)
                oeng.tensor_scalar(o4[(slice(None), 0, 0) + sl[1:]], w_[sl], 1086.5 - C, step, Alu.min, Alu.mult)
                off += wd
            if gctx:
                gctx.__exit__(None, None, None)
            goff += gw

        off = 0
        for k, (wd, eng) in enumerate(zip(out_segs, out_eng)):
            sl = (slice(None), slice(off, off + wd))
            if eng == "k":
                # pool-prepared descriptors + cheap trigger: the trigger
                # carries the data dependency and skips HWDGE + DGE delay
                in4 = o4[(slice(None), slice(None), slice(None)) + sl[1:]]  # [P,1,1,wd]
                prep = nc.gpsimd.kv_writeback(
                    o_d4, in4, kv_idx[k][:], prepare_only=True, sem=kv_sem
                )
                # Drop the wrapper-added completion inc so tile's DMASW sem
                # becomes on_update[0]: both sims defer slot 0 to the
                # trigger, and tile's epilogue waits on DMASW — giving the
                # true transfer-completion semantics with no extra wait.
                prep.ins.sync_info = mybir.SyncInfo(on_wait=[], on_update=[])
                nc.gpsimd.trigger_dma(count=None)
            else:
                _engine(nc, eng).dma_start(o_d4[(0, 0) + sl], o4[(slice(None), 0, 0) + sl[1:]])
            off += wd

    nc.compile()
    return nc


def _freeze(obj):
    if isinstance(obj, dict):
        return tuple(sorted((k, _freeze(v)) for k, v in obj.items()))
    if isinstance(obj, (list, tuple)):
        return tuple(_freeze(v) for v in obj)
    return obj


def build(bins: np.ndarray, plan=None):
    key = _constants(bins)
    if key is None:
        raise NotImplementedError("bins not supported by this kernel")
    full_key = (key, _freeze(plan))
    if full_key not in _BUILD_CACHE:
        _BUILD_CACHE[full_key] = _build(*key, plan=plan)
    return _BUILD_CACHE[full_key]


def make_in_maps(Xs: np.ndarray):
    shards = Xs.reshape(NCORES, P, W)
    return [{"x": shards[c]} for c in range(NCORES)]


def kernel(Xs: np.ndarray, bins: np.ndarray) -> np.ndarray:
    Xs = np.asarray(Xs, dtype=np.float32)
    bins = np.asarray(bins, dtype=np.float32)
    nc = build(bins)
    res = run_bass_kernel_spmd(nc, make_in_maps(Xs), core_ids=list(range(NCORES)))
    out = np.concatenate([r["o"].reshape(-1) for r in res.results])
    return out.astype(np.float32)


# revision 5
# speedup vs baseline: 1.0274x; 1.0030x over previous
"""Trainium2 Bass kernel v2 for nn_LogOddsPerformanceTransformer.

For each element x:  s = logit(x);  out = bins[clip(floor((s-b0)/step),0,63)]

Post-ACT arithmetic runs in fp16: tensor_scalar gets the DVE 4x perf
mode and the output DMA halves (values round to fp16; norm-rel ~5e-3,
well under the 2e-2 gate).  Magic-number floor in fp16 (1024 has ulp 1):
    t1 = round16(s*inv + C)        -> 1024 + floor(g),  g=(s-b0)/step
    w  = max(t1 - (C+0.5), 1023.5-C)   (f32 scalars; exact on 0.5 grid)
    o  = min(w, 1086.5-C) * step       (single fp16 rounding)

Two modes per column group:
  A: a=Ln(x), b=Ln(1-x) on ACT (fp16 out); s=a-b (TT on DVE 2x or Pool)
  B: r=reciprocal(x) on DVE (f32); s'=Ln(r-1) on ACT via bias AP=-1
     (fp16); the sign of s' folds into -inv.  One ACT pass instead of
     two — B groups go last so the drain isn't gated on a busy ACT.

The plan decouples granularities: fine input DMA segments keep the ACT
ramp fed; ACT instructions are coarse (222-cycle init each); TT/TS/out
run on sub-chunks for smooth downstream cadence and early outs.

Data parallel over 8 cores; per core [128 x 4096] f32 in, fp16 out,
single DRAM tensors, slice DMAs, full-width SBUF stage buffers.
"""

import sys

sys.path.insert(0, "/opt/trn_rl_repo")

from contextlib import ExitStack

import numpy as np

import concourse.bass as bass
import concourse.tile as tile
from concourse import bacc, mybir
from concourse.bass_utils import run_bass_kernel_spmd

N = 4_194_304
NCORES = 8
NPER = N // NCORES  # 524288
P = 128
W = NPER // P  # 4096 columns per core

# --- plan -----------------------------------------------------------------
# groups: mode 'A'|'B'; cols = ACT instruction span; sub = TT/TS/out chunk
# widths within the group; tt: 'v' DVE / 'p' Pool (A only); w_eng 'v'|'p'
# per-sub engine for the w stage.
PLAN = dict(
    in_segs=(256, 512, 384, 384, 512, 512, 512, 512, 512),
    in_eng=("s",) * 9,
    groups=(
        dict(mode="A", sub=(256,), tt="v", w_eng=("p",), o_eng=("p",)),
        dict(mode="A", sub=(512,), tt="v", w_eng=("p",), o_eng=("p",)),
        dict(mode="A", sub=(384, 384), tt="v", w_eng=("p", "p"), o_eng=("p", "p")),
        dict(mode="B", sub=(1024,), recip=(512, 512)),
        dict(mode="B", sub=(1024,), recip=(512, 512)),
        dict(mode="B", sub=(512,), recip=(512,)),
    ),
    out_segs=(256, 512, 384, 384, 1024, 1024, 512),
    out_eng=("s",) * 7,
)
# --------------------------------------------------------------------------

f32 = mybir.dt.float32
f16 = mybir.dt.float16
Alu = mybir.AluOpType
Act = mybir.ActivationFunctionType

_BUILD_CACHE: dict[tuple, object] = {}


def _constants(bins: np.ndarray):
    b64 = bins.astype(np.float64)
    nb = len(bins)
    if nb != 64:
        return None
    step = np.float32((b64[-1] - b64[0]) / (nb - 1))
    inv = np.float32((nb - 1) / (b64[-1] - b64[0]))
    # C = 1024 + (-b0*inv - 0.5): the fp16 round of s*inv + C floors g.
    # For linspace(-6,6,64): -b0*inv = 31.5 so C = 1055.0 exactly.
    C = 1024.0 - float(b64[0]) * float(inv) - 0.5
    if C != float(np.float32(C)) or not (1024.0 < C < 1088.0):
        return None
    uniform = np.allclose(np.diff(b64), (b64[-1] - b64[0]) / (nb - 1), rtol=0, atol=1e-5)
    if not uniform:
        return None
    return (float(step), float(inv), C)


def _engine(nc, code):
    return {"s": nc.sync, "v": nc.vector, "p": nc.gpsimd, "a": nc.scalar}[code]


def _build(step, inv, C, plan=None):
    plan = plan or PLAN
    groups = plan["groups"]
    in_segs = plan["in_segs"]
    out_segs = plan["out_segs"]
    in_eng = plan.get("in_eng", ("s",) * len(in_segs))
    out_eng = plan.get("out_eng", ("s",) * len(out_segs))
    gcols = [sum(g["sub"]) for g in groups]
    assert sum(gcols) == W, (sum(gcols), W)
    assert sum(e[1] if isinstance(e, tuple) else e for e in in_segs) == W
    assert sum(out_segs) == W

    nc = bacc.Bacc("TRN2", target_bir_lowering=False, debug=False)
    x_d = nc.dram_tensor("x", [P, W], f32, kind="ExternalInput").ap()
    # 4-D [batch=1, dhi=1, dho=P, n_ctx=W] so kv_writeback can address it;
    # plain DMA outs use o_d4[0, 0] slices.
    o_d4 = nc.dram_tensor("o", [1, 1, P, W], f16, kind="ExternalOutput").ap()
    n_kv = sum(1 for e in out_eng if e == "k")
    kv_sem = nc.alloc_semaphore("kv_out_sem") if n_kv else None

    with tile.TileContext(nc) as tc, ExitStack() as ctx:
        pool = ctx.enter_context(tc.tile_pool(name="pool", bufs=1))

        need_b = any(g["mode"] == "B" for g in groups)
        cm1 = pool.tile([P, 1], f32, tag="cm1")
        nc.gpsimd.memset(cm1[:], -1.0)
        # Dummy 1-col Ln emitted before any DMA: insert_act_table_loads
        # places the 1283ns natural_log table load here, during the DMA
        # ramp, instead of gating the first real activation on it.
        warm = pool.tile([P, 1], f16, tag="warm")
        nc.scalar.activation(warm[:], cm1[:], Act.Ln, 1.0, -1.0)

        x = pool.tile([P, W], f32, tag="x")
        a = pool.tile([P, W], f16, tag="a")
        b = pool.tile([P, W], f16, tag="b")
        r = pool.tile([P, W], f32, tag="r")
        s = pool.tile([P, W], f16, tag="s")
        t1 = pool.tile([P, W], f16, tag="t1")
        w_ = pool.tile([P, W], f16, tag="w")
        o4 = pool.tile([P, 1, 1, W], f16, tag="o")

        # column-offset index tiles for the kv outs, memset early
        kv_idx = {}
        off = 0
        for k, (wd, eng) in enumerate(zip(out_segs, out_eng)):
            if eng == "k":
                iw = pool.tile([P, 1], mybir.dt.int32, tag=f"oidx{k}")
                nc.gpsimd.memset(iw[:], off)
                kv_idx[k] = iw
            off += wd

        # in_segs entries: width (sequential) or (col_offset, width) for an
        # explicit transfer order — the DMA queue order is free even though
        # column ranges are fixed
        segs = []
        off = 0
        for ent in in_segs:
            if isinstance(ent, tuple):
                segs.append(ent)
            else:
                segs.append((off, ent))
                off += ent
        cov = sorted(segs)
        assert cov[0][0] == 0 and all(
            a + w == b for (a, w), (b, _) in zip(cov, cov[1:])
        ) and cov[-1][0] + cov[-1][1] == W, f"in_segs don't tile [0,{W}): {cov}"
        with tc.high_priority():
            for (start, wd), eng in zip(segs, in_eng):
                sl = (slice(None), slice(start, start + wd))
                _engine(nc, eng).dma_start(x[sl], x_d[sl])

        goff = 0
        for g in groups:
            gctx = tc.high_priority(offset=g["prio"]) if g.get("prio") else None
            if gctx:
                gctx.__enter__()
            gw = sum(g["sub"])
            gsl = (slice(None), slice(goff, goff + gw))
            if g["mode"] == "A":
                nc.scalar.activation(a[gsl], x[gsl], Act.Ln)
                nc.scalar.activation(b[gsl], x[gsl], Act.Ln, 1.0, -1.0)
            else:
                roff = goff
                for rw in g["recip"]:
                    rsl = (slice(None), slice(roff, roff + rw))
                    nc.vector.reciprocal(r[rsl], x[rsl])
                    roff += rw
                # s' = Ln(r - 1) = -s ; sign folds into -inv below
                nc.scalar.activation(s[gsl], r[gsl], Act.Ln, cm1[:, 0:1])
            off = goff
            for i, wd in enumerate(g["sub"]):
                sl = (slice(None), slice(off, off + wd))
                if g["mode"] == "A":
                    eng = nc.gpsimd if g.get("tt") == "p" else nc.vector
                    eng.tensor_tensor(s[sl], a[sl], b[sl], Alu.subtract)
                    nc.vector.tensor_scalar(t1[sl], s[sl], inv, C, Alu.mult, Alu.add)
                else:
                    nc.vector.tensor_scalar(t1[sl], s[sl], -inv, C, Alu.mult, Alu.add)
                w_engs = g.get("w_eng")
                weng = nc.gpsimd if (w_engs and w_engs[i] == "p") else nc.vector
                o_engs = g.get("o_eng")
                oeng = nc.gpsimd if (o_engs and o_engs[i] == "p") else nc.vector
                # w = max(t1 - (1024+b0i), -b0i); o = min(w, 63-b0i) * step
                # with b0i = C + 0.5 - 1024 (f32 scalars, exact 0.5-grid out)
                weng.tensor_scalar(w_[sl], t1[sl], C + 0.5, 1023.5 - C, Alu.subtract, Alu.max)
                oeng.tensor_scalar(o4[(slice(None), 0, 0) + sl[1:]], w_[sl], 1086.5 - C, step, Alu.min, Alu.mult)
                off += wd
            if gctx:
                gctx.__exit__(None, None, None)
            goff += gw

        off = 0
        for k, (wd, eng) in enumerate(zip(out_segs, out_eng)):
            sl = (slice(None), slice(off, off + wd))
            if eng == "k":
                # pool-prepared descriptors + cheap trigger: the trigger
                # carries the data dependency and skips HWDGE + DGE delay
                in4 = o4[(slice(None), slice(None), slice(None)) + sl[1:]]  # [P,1,1,wd]
                prep = nc.gpsimd.kv_writeback(
                    o_d4, in4, kv_idx[k][:], prepare_only=True, sem=kv_sem
                )
                # Drop the wrapper-added completion inc so tile's DMASW sem
                # becomes on_update[0]: both sims defer slot 0 to the
                # trigger, and tile's epilogue waits on DMASW — giving the
                # true transfer-completion semantics with no extra wait.
                prep.ins.sync_info = mybir.SyncInfo(on_wait=[], on_update=[])
                nc.gpsimd.trigger_dma(count=None)
            else:
                _engine(nc, eng).dma_start(o_d4[(0, 0) + sl], o4[(slice(None), 0, 0) + sl[1:]])
            off += wd

    nc.compile()
    return nc


def _freeze(obj):
    if isinstance(obj, dict):
        return tuple(sorted((k, _freeze(v)) for k, v in obj.items()))
    if isinstance(obj, (list, tuple)):
        return tuple(_freeze(v) for v in obj)
    return obj


def build(bins: np.ndarray, plan=None):
    key = _constants(bins)
    if key is None:
        raise NotImplementedError("bins not supported by this kernel")
    full_key = (key, _freeze(plan))
    if full_key not in _BUILD_CACHE:
        _BUILD_CACHE[full_key] = _build(*key, plan=plan)
    return _BUILD_CACHE[full_key]


def make_in_maps(Xs: np.ndarray):
    shards = Xs.reshape(NCORES, P, W)
    return [{"x": shards[c]} for c in range(NCORES)]


def kernel(Xs: np.ndarray, bins: np.ndarray) -> np.ndarray:
    Xs = np.asarray(Xs, dtype=np.float32)
    bins = np.asarray(bins, dtype=np.float32)
    nc = build(bins)
    res = run_bass_kernel_spmd(nc, make_in_maps(Xs), core_ids=list(range(NCORES)))
    out = np.concatenate([r["o"].reshape(-1) for r in res.results])
    return out.astype(np.float32)
